# revision 24
# baseline (speedup 1.0000x reference)
"""Sliding-window causal GQA attention with ALiBi for Trainium2, SPMD on 8
NeuronCores.

Problem (hardcoded): B=1, S=2048, D=2048, 16 query heads / 4 KV groups,
head_dim 128, window 512.

Sharding: tensor parallel over heads — core c owns KV group c//2 and query
head pair c%2 within that group (2 query heads per core, full sequence).
Wq/Wk/Wv are column-sharded by head, Wo row-sharded; each core produces a
full-shape partial of the output projection and the host sums the 8 partials
(replaces the all-reduce).

Device-side layout: the host passes x TRANSPOSED (xt = x.T, [D, S]). All
projections then emit transposed activations (qT/kT/vT = [head_dim, S]),
scores are computed as [keys, q] blocks — exactly the operand order the PE
array wants for the probs @ V matmul (keys on the contraction partition) —
and yT = [head_dim, q] is exactly the lhsT the output projection wants. The
V tiles are turned into [key, head_dim] layout by 16 SBUF->SBUF xbar DMA
transposes (no PE/DVE time).

Pipeline structure (v1 profile showed: scalar queue 112us busy, sync queue
91us issuing 170 small DMAs, PE 65% occupied and HAM-throttled; each fix
below removed a measured stall):
 - every weight arrives as ONE large DMA into one contiguous SBUF tile; x
   arrives per-chunk on the two HWDGE queues, with the first-matmul prefix
   (wk chunk 0, first quarter of x chunk 0, wv) split out small so the PE
   starts ~5us earlier.
 - phase 1a interleaves the K and V projections per contraction chunk
   (8 PSUM banks) so PE consumption (~1.7us/chunk) outpaces DMA arrival
   and the PE never goes cold waiting on x; the 16 V-tile xbar DMA
   transposes then run on the sync queue underneath phase 1b (Q0+Q1,
   sc-serial so qt chunk 0 is drained long before attention reads it).
 - attention is software-pipelined: block i+1's score/bias matmuls issue
   before block i's PV/rowsum, so the in-order PE queue never stalls on the
   scalar-engine exp.
 - the softmax row-sum is accumulated with an all-ones [128,128] stationary
   operand so the PSUM result is replicated across all 128 partitions;
   1/rowsum is 4 plain DVE ops (magic-seed + one Newton step, sign carried
   through two fused scalar_tensor_tensor ops) and the normalize multiply
   reads y straight out of PSUM — no DRAM bounce, no 3.3us reciprocal.
 - out-proj is emitted in halves between the two heads' attention (drain
   copies fill scalar/DVE slack), pairs matmuls per yt stationary to halve
   LDWEIGHTS switches, and stores bf16 via split DMAs on both HWDGE queues
   with a 4-deep stage pool.

Softmax: scores are small (|qk/sqrt(d)| ~ 4) and the ALiBi bias negative, so
fp32 exp never overflows and the max-subtraction pass is skipped. The
window/causal mask + ALiBi bias live in a host-precomputed [128, 640]
template added to the scores PSUM via an identity matmul on the PE.
"""

import math

import numpy as np
import ml_dtypes

import concourse.bass as bass
import concourse.mybir as mybir
import concourse.tile as tile
from concourse.masks import make_identity

BF16 = ml_dtypes.bfloat16

B, S, D = 1, 2048, 2048
NH, NKV, HD = 16, 4, 128
REP = NH // NKV          # query heads per KV group
WINDOW = 512
NCORES = 8
HPC = 2                  # query heads per core
QC = 512                 # q-chunk width (one PSUM bank of fp32)
NQC = S // QC            # 4
NKT = S // 128           # 16 key tiles
NDC = D // 128           # 16 contraction chunks
TW = WINDOW + 128        # 640: bias template width
NEG = -1.0e30

FP32 = mybir.dt.float32
BF = mybir.dt.bfloat16


def _alibi_slopes(n_heads: int) -> np.ndarray:
    def pow2_slopes(n):
        start = 2.0 ** (-(2.0 ** (-(math.log2(n) - 3))))
        return [start * start**i for i in range(n)]

    if math.log2(n_heads).is_integer():
        slopes = pow2_slopes(n_heads)
    else:
        closest = 2 ** math.floor(math.log2(n_heads))
        slopes = pow2_slopes(closest)
        slopes += pow2_slopes(2 * closest)[0::2][: n_heads - closest]
    return np.asarray(slopes, dtype=np.float32)


def _bias_templates() -> np.ndarray:
    """[NH, 128, TW] fp32. Template col c of key-tile row kc corresponds to
    query position q = k0 + c (k0 = key tile start). Valid iff kc <= c <=
    kc + WINDOW - 1; value -slope * (c - kc), else -1e30."""
    slopes = _alibi_slopes(NH)
    kc = np.arange(128)[:, None]
    c = np.arange(TW)[None, :]
    dist = (c - kc).astype(np.float32)
    valid = (dist >= 0) & (dist <= WINDOW - 1)
    out = np.empty((NH, 128, TW), np.float32)
    for h in range(NH):
        out[h] = np.where(valid, -slopes[h] * dist, NEG)
    return out


def _split_waits(nc, maxw=1):
    """This container's walrus rejects instructions with more than one sync
    wait command; hoist extra waits onto preceding same-engine NoOps."""
    plan = {}
    si_type = None
    for bb in nc.main_func.blocks:
        for ins in bb.instructions:
            si = ins.sync_info
            waits = list(si.on_wait) if si and si.on_wait else []
            if len(waits) > maxw:
                si_type = type(si)
                extra = [waits[i:i + maxw] for i in range(0, len(waits) - maxw, maxw)]
                keep = waits[len(extra) * maxw:]
                plan[ins.name] = (extra, keep)
    if not plan:
        return 0
    nops = {}
    nop_names = set()
    for name, (extra, _keep) in plan.items():
        target = nc.inst_map[name]
        eng = nc.engines[target.engine]
        lst = []
        for chunk in extra:
            nop = eng.nop(nofuse=True).ins
            nop.sync_info = si_type(on_wait=chunk, on_update=[])
            lst.append(nop)
            nop_names.add(nop.name)
        nops[name] = lst
    for bb in nc.main_func.blocks:
        insts = list(bb.instructions)
        out = []
        changed = False
        for ins in insts:
            if ins.name in nop_names:
                changed = True
                continue
            if ins.name in plan:
                _extra, keep = plan[ins.name]
                si = ins.sync_info
                upd = list(si.on_update) if si and si.on_update else []
                ins.sync_info = si_type(on_wait=keep, on_update=upd)
                out.extend(nops[ins.name])
                changed = True
            out.append(ins)
        if changed:
            bb.instructions = out
    return len(plan)


def _kt_range(qc):
    """Key tiles feeding q-chunk qc: keys [qc*QC - WINDOW + 1, qc*QC + QC - 1]."""
    lo = max(0, (qc * QC - WINDOW + 1) // 128)
    hi = (qc * QC + QC - 1) // 128
    return lo, hi


def _build_program():
    nc = bass.Bass()

    # weight/bias inputs arrive pre-shuffled to partition-major layouts so
    # every input DMA is fully contiguous per partition row
    xt = nc.dram_tensor("xt", [D, S], BF, kind="ExternalInput")
    wq = nc.dram_tensor("wq", [128, NDC * HPC * HD], BF, kind="ExternalInput")
    wk = nc.dram_tensor("wk", [128, NDC * HD], BF, kind="ExternalInput")
    wv = nc.dram_tensor("wv", [128, NDC * HD], BF, kind="ExternalInput")
    wo = nc.dram_tensor("wo", [128, HPC * D], BF, kind="ExternalInput")
    biast = nc.dram_tensor("biast", [128, HPC * TW], BF, kind="ExternalInput")
    out = nc.dram_tensor("out", [S, D], BF, kind="ExternalOutput")

    with tile.TileContext(nc) as tc:
        with tc.tile_pool(name="persist", bufs=1) as persist:
            xt_sb = [persist.tile([128, S], BF, name=f"xt{d}") for d in range(NDC)]
            wq_sb = persist.tile([128, NDC * HPC * HD], BF)
            wk_sb = persist.tile([128, NDC * HD], BF)
            wv_sb = persist.tile([128, NDC * HD], BF)
            wo_sb = persist.tile([128, HPC, D], BF)
            bias_sb = persist.tile([128, HPC, TW], BF)
            qt_sb = [persist.tile([128, S], BF, name=f"qt{h}") for h in range(HPC)]
            kt_sb = persist.tile([128, S], BF)
            vt_sb = persist.tile([128, S], BF)
            v_sb = persist.tile([128, NKT, HD], BF)
            # normalized y^T per (h, qc): [hd, q]
            yt_sb = persist.tile([128, HPC, S], BF)
            ident = persist.tile([128, 128], BF)
            ones_sq = persist.tile([128, 128], BF)

            # input DMAs: one large descriptor per weight, x per-chunk on
            # alternating HWDGE queues (completion order = program order per
            # queue). The K/V-blocking prefix (wk chunk 0, first quarter of
            # x chunk 0, wv) is split out small so the first matmul issues
            # ~5us earlier instead of waiting behind 512KB transfers.
            nc.sync.dma_start(out=wk_sb[:, :HD], in_=wk[:, :HD])
            nc.sync.dma_start(out=xt_sb[0][:, :QC], in_=xt[0:128, :QC])
            nc.scalar.dma_start(out=wv_sb[:, :HD], in_=wv[:, :HD])
            nc.scalar.dma_start(out=wv_sb[:, HD:], in_=wv[:, HD:])
            nc.sync.dma_start(out=wk_sb[:, HD:], in_=wk[:, HD:])
            nc.sync.dma_start(out=xt_sb[0][:, QC:], in_=xt[0:128, QC:])
            for dch in range(1, NDC):
                if dch in (3, 5):
                    eng = nc.gpsimd
                elif dch % 2 == 0:
                    eng = nc.sync
                else:
                    eng = nc.scalar
                eng.dma_start(out=xt_sb[dch], in_=xt[dch * 128:(dch + 1) * 128, :])
            nc.scalar.dma_start(out=wq_sb, in_=wq[:, :])
            nc.sync.dma_start(out=wo_sb.rearrange("p h n -> p (h n)"), in_=wo[:, :])
            nc.scalar.dma_start(out=bias_sb.rearrange("p h c -> p (h c)"), in_=biast[:, :])
            make_identity(nc, ident)
            nc.vector.memset(ones_sq, 1.0)

            # ---- phase 1: projections (all emitted transposed) ----
            # Two interleaved projections at a time = 8 PSUM banks; PE burns
            # 4096 cols (~1.7us) per x chunk, above the ~1.4us DMA arrival
            # rate, so phase 1a is DMA-overlapped and 1b runs from SBUF.
            with tc.tile_pool(name="proj_ps", bufs=8, space="PSUM") as proj_ps:
                # 1a: K + V interleaved per contraction chunk (overlaps the x
                # DMA); both are needed before attention's PV matmuls, and
                # doing them first lets the V transposes run on the DMA
                # queues underneath phase 1b.
                pss = [
                    [
                        proj_ps.tile([128, QC], FP32, tag="proj", name=f"pp{si}_{sc}")
                        for sc in range(NQC)
                    ]
                    for si in range(2)
                ]
                for dch in range(NDC):
                    for si, w_sb in enumerate((wk_sb, wv_sb)):
                        for sc in range(NQC):
                            nc.tensor.matmul(
                                pss[si][sc],
                                w_sb[:, dch * HD:(dch + 1) * HD],
                                xt_sb[dch][:, sc * QC:(sc + 1) * QC],
                                start=(dch == 0),
                                stop=(dch == NDC - 1),
                                skip_group_check=True,
                            )
                for sc in range(NQC):
                    nc.vector.tensor_copy(kt_sb[:, sc * QC:(sc + 1) * QC], pss[0][sc])
                    nc.scalar.copy(
                        out=vt_sb[:, sc * QC:(sc + 1) * QC], in_=pss[1][sc]
                    )
                # V tiles -> [key, head_dim] via xbar DMA transpose on the
                # sync queue (x loads are done by now; the scalar queue must
                # stay clear so wq/wo/biast finish early). They complete
                # under phase 1b, so attention never waits on them.
                for kt in range(NKT):
                    nc.sync.dma_start_transpose(
                        v_sb[:, kt, :], vt_sb[:, kt * 128:(kt + 1) * 128]
                    )
                # 1b: Q0 + Q1 from resident x, sc-serial so qt0's chunk 0 is
                # copied out ~20us before attention needs it (no boundary
                # stall on the PSUM->SBUF copies).
                for h in range(HPC):
                    for sc in range(NQC):
                        ps = proj_ps.tile([128, QC], FP32, tag="proj", name=f"q{h}_{sc}")
                        for dch in range(NDC):
                            nc.tensor.matmul(
                                ps,
                                wq_sb[:, dch * HPC * HD + h * HD:dch * HPC * HD + (h + 1) * HD],
                                xt_sb[dch][:, sc * QC:(sc + 1) * QC],
                                start=(dch == 0),
                                stop=(dch == NDC - 1),
                            )
                        if sc % 2 == 0:
                            nc.vector.tensor_copy(
                                qt_sb[h][:, sc * QC:(sc + 1) * QC], ps
                            )
                        else:
                            nc.scalar.copy(
                                out=qt_sb[h][:, sc * QC:(sc + 1) * QC], in_=ps
                            )

            # ---- phase 2: attention + output projection, per q-chunk ----
            with tc.tile_pool(name="sc_ps", bufs=2, space="PSUM") as sc_ps, \
                 tc.tile_pool(name="yt_ps", bufs=2, space="PSUM") as yt_ps, \
                 tc.tile_pool(name="rs_ps", bufs=2, space="PSUM") as rs_ps, \
                 tc.tile_pool(name="op_ps", bufs=2, space="PSUM") as op_ps, \
                 tc.tile_pool(name="et_sb", bufs=4) as et_pool, \
                 tc.tile_pool(name="rc_sb", bufs=3) as rc_pool, \
                 tc.tile_pool(name="stage_sb", bufs=4) as stage_pool:

                def attention(h, qc):
                    q0 = qc * QC
                    klo, khi = _kt_range(qc)
                    y_ps = yt_ps.tile([128, QC], FP32, tag="y")
                    r_ps = rs_ps.tile([128, QC], FP32, tag="r")
                    # shifted-window PSUM accumulation: the first matmul
                    # (start=True) must cover all 512 columns since
                    # has_written is per-element; key tile 4*qc always does.
                    kts = [4 * qc] + [t for t in range(klo, khi + 1) if t != 4 * qc]

                    def pv_rs(kt, q_lo, w, first, last, et):
                        nc.tensor.matmul(
                            y_ps[:, q_lo - q0:q_lo - q0 + w],
                            v_sb[:, kt, :],
                            et[:, :w],
                            start=first,
                            stop=last,
                            skip_group_check=True,
                        )
                        # all-ones stationary operand -> row-sum replicated
                        # across all 128 PSUM partitions (feeds a parallel
                        # DVE reciprocal + broadcast-free normalize)
                        nc.tensor.matmul(
                            r_ps[:, q_lo - q0:q_lo - q0 + w],
                            ones_sq,
                            et[:, :w],
                            start=first,
                            stop=last,
                            skip_group_check=True,
                        )

                    # software pipeline: PV/rowsum of block i issue AFTER
                    # score/bias of block i+1, so the in-order PE queue never
                    # stalls waiting for the scalar-engine exp of block i.
                    pending = None
                    for i, kt in enumerate(kts):
                        k0 = kt * 128
                        q_lo = max(q0, k0)
                        q_hi = min(q0 + QC - 1, k0 + TW - 1)
                        w = q_hi - q_lo + 1
                        first, last = i == 0, i == len(kts) - 1
                        s_ps = sc_ps.tile([128, QC], FP32, tag="sc")
                        nc.tensor.matmul(
                            s_ps[:, :w],
                            kt_sb[:, kt * 128:kt * 128 + 128],
                            qt_sb[h][:, q_lo:q_hi + 1],
                            start=True,
                            stop=False,
                        )
                        nc.tensor.matmul(
                            s_ps[:, :w],
                            ident,
                            bias_sb[:, h, q_lo - k0:q_lo - k0 + w],
                            start=False,
                            stop=True,
                        )
                        et = et_pool.tile([128, QC], BF, tag="et")
                        nc.scalar.activation(
                            out=et[:, :w],
                            in_=s_ps[:, :w],
                            func=mybir.ActivationFunctionType.Exp,
                        )
                        if pending is not None:
                            pv_rs(*pending)
                        pending = (kt, q_lo, w, first, last, et)
                    pv_rs(*pending)
                    # 1/rowsum in 4 plain DVE ops (the fused custom-DVE
                    # reciprocal doesn't compile on this walrus): magic-number
                    # seed (max rel err 3.4%) + one Newton-Raphson step
                    # (-> 1.2e-3 max), sign carried through the last two fused
                    # scalar_tensor_tensor ops. rowsums are positive normals
                    # so the bit trick is safe.
                    y0 = rc_pool.tile([128, QC], mybir.dt.int32, tag="y0")
                    nc.vector.tensor_scalar(
                        out=y0,
                        in0=r_ps.bitcast(mybir.dt.int32),
                        scalar1=-1,
                        scalar2=0x7EF311C3,
                        op0=mybir.AluOpType.mult,
                        op1=mybir.AluOpType.add,
                    )
                    y0f = y0.bitcast(FP32)
                    t = rc_pool.tile([128, QC], FP32, tag="t")
                    nc.vector.tensor_tensor(t, r_ps, y0f, mybir.AluOpType.mult)
                    negy1 = rc_pool.tile([128, QC], FP32, tag="ny")
                    nc.vector.scalar_tensor_tensor(
                        negy1, t, 2.0, y0f,
                        mybir.AluOpType.subtract, mybir.AluOpType.mult,
                    )
                    nc.vector.scalar_tensor_tensor(
                        yt_sb[:, h, q0:q0 + QC], y_ps, -1.0, negy1,
                        mybir.AluOpType.mult, mybir.AluOpType.mult,
                    )

                def outproj(qc, stis):
                    # ncol pairs share each yt lhsT across 2 consecutive
                    # matmuls (halves the LDWEIGHTS switch rate); the two
                    # stage-half store DMAs go to different HWDGE queues so
                    # the final drain isn't serialized on one queue.
                    for sti in stis:
                        st = qc * 4 + sti
                        stage = stage_pool.tile([128, D], BF, tag="stg")
                        for ncp in range(2):
                            pss = [op_ps.tile([128, QC], FP32, tag="op",
                                              name=f"op{j}") for j in range(2)]
                            for h in range(HPC):
                                for j in range(2):
                                    nc.tensor.matmul(
                                        pss[j],
                                        yt_sb[:, h, st * 128:(st + 1) * 128],
                                        wo_sb[:, h, (2 * ncp + j) * QC:(2 * ncp + j + 1) * QC],
                                        start=(h == 0),
                                        stop=(h == HPC - 1),
                                        skip_group_check=True,
                                    )
                            for j in range(2):
                                ncol = 2 * ncp + j
                                if ncol % 2 == 0:
                                    nc.scalar.copy(
                                        out=stage[:, ncol * QC:(ncol + 1) * QC],
                                        in_=pss[j],
                                    )
                                else:
                                    nc.vector.tensor_copy(
                                        stage[:, ncol * QC:(ncol + 1) * QC], pss[j]
                                    )
                        nc.sync.dma_start(
                            out=out[st * 128:(st + 1) * 128, :D // 2],
                            in_=stage[:, :D // 2],
                        )
                        half2 = nc.gpsimd if qc == NQC - 1 else nc.scalar
                        half2.dma_start(
                            out=out[st * 128:(st + 1) * 128, D // 2:],
                            in_=stage[:, D // 2:],
                        )

                # out-proj lags attention by one q-chunk (the normalize
                # chain's latency never backs up the PE stream) and is
                # emitted in halves between the two heads so its PSUM-drain
                # copies fill the scalar/DVE queues' slack instead of
                # queueing ahead of the next chunk's exps.
                for qc in range(NQC):
                    attention(0, qc)
                    if qc > 0:
                        outproj(qc - 1, [0, 1])
                    attention(1, qc)
                    if qc > 0:
                        outproj(qc - 1, [2, 3])
                outproj(NQC - 1, [0, 1, 2, 3])

    _split_waits(nc, maxw=1)
    return nc


_NC_CACHE = None


def _get_program():
    global _NC_CACHE
    if _NC_CACHE is None:
        _NC_CACHE = _build_program()
    return _NC_CACHE


def _shuffle_chunks(w, cols):
    """[D, cols] -> [128, NDC*cols] partition-major contiguous layout."""
    return np.ascontiguousarray(
        w.reshape(NDC, 128, cols).transpose(1, 0, 2).reshape(128, NDC * cols)
    )


def build_in_maps(x, Wq, Wk, Wv, Wo):
    x = np.asarray(x, np.float32)
    Wq = np.asarray(Wq, np.float32)
    Wk = np.asarray(Wk, np.float32)
    Wv = np.asarray(Wv, np.float32)
    Wo = np.asarray(Wo, np.float32)

    xt = np.ascontiguousarray(x[0].T).astype(BF16)
    wq_s = (Wq * (1.0 / math.sqrt(HD))).astype(BF16)
    wk_s = Wk.astype(BF16)
    wv_s = Wv.astype(BF16)
    wo_s = Wo.astype(BF16)
    templates = _bias_templates()

    in_maps = []
    for c in range(NCORES):
        g, hp = c // HPC, c % HPC
        heads = [g * REP + hp * HPC + r for r in range(HPC)]
        wo_rows = wo_s[heads[0] * HD:(heads[-1] + 1) * HD, :]  # [256, D]
        in_maps.append(
            {
                "xt": xt,
                "wq": _shuffle_chunks(
                    wq_s[:, heads[0] * HD:(heads[-1] + 1) * HD], HPC * HD
                ),
                "wk": _shuffle_chunks(wk_s[:, g * HD:(g + 1) * HD], HD),
                "wv": _shuffle_chunks(wv_s[:, g * HD:(g + 1) * HD], HD),
                "wo": np.ascontiguousarray(
                    wo_rows.reshape(HPC, 128, D).transpose(1, 0, 2).reshape(128, HPC * D)
                ),
                "biast": np.ascontiguousarray(
                    templates[heads].transpose(1, 0, 2).reshape(128, HPC * TW)
                ).astype(BF16),
            }
        )
    return in_maps


_last_in_maps = None


def kernel(x, Wq, Wk, Wv, Wo):
    from concourse.bass_utils import run_bass_kernel_spmd

    global _last_in_maps
    in_maps = build_in_maps(x, Wq, Wk, Wv, Wo)
    _last_in_maps = in_maps

    nc = _get_program()
    res = run_bass_kernel_spmd(nc, in_maps, list(range(NCORES)))
    acc = res.results[0]["out"].astype(np.float64)
    for c in range(1, NCORES):
        acc += res.results[c]["out"].astype(np.float64)
    return acc.astype(np.float32).reshape(B, S, D)
